# revision 3
# baseline (speedup 1.0000x reference)
"""TopK-SAE on 8 TRN2 cores — v5: fp8 DoubleRow encode + host-P decode.

Launch 1 (dict-sharded): z0 = fp8e4 DoubleRow encode (W pre-scaled by 64,
fp32 PSUM accumulation), per-dict-row top-8 values + batch indices from
PSUM -> per-core candidate tables. Runs at the fp8 PE roofline (~155
TF/s/core). DMA issue order prioritizes x-chunk 0 + first two weight
tiles; the first two d-tiles run batch-major so compute starts before
the full x broadcast lands.
Host: exact merge — union of candidates with noisy value >= kth0-DELTA
re-dotted (fp32 einsum + fp64 inside a tight boundary window), exact
global top-K; latents sorted by batch row, packed into 16 chunks of 128
batch rows x 3 slot groups (384 slots/chunk; host applies any overflow
latents directly to the output).
Launch 2 (A-sharded): x_hatT slice += G_g.T @ P_g per slot group, with
the P one-hot/act matrices prebuilt on host and DMAed (no DVE mask
build). fp16 output, upcast on host.
"""
import numpy as np

B, A, D, K = 2048, 4096, 32768, 4096
NCORES = 8
DL = D // NCORES            # dict rows per core
DT = DL // 128              # d-tiles per core
KT2 = A // 256              # DoubleRow contraction tiles
BCH = 512                   # encode matmul free-dim chunk
NBCH = B // BCH
DCH = 128                   # decode batch chunk (rows)
NDCH = B // DCH             # 16 decode chunks
NGC = 3                     # slot groups per decode chunk
CAPC = NGC * 128            # 384 slots per chunk
NSLOT = NDCH * CAPC         # 6144 total slots
NGRP = NSLOT // 128         # 48 slot groups
AS = A // NCORES            # A-shard per core
AT = AS // 128
WSCALE = 64.0
DELTA = 0.25                # fp8 z0 noise band (sigma ~0.0375, max ~0.21)
FP64_WND = 0.002            # fp64 re-dot window around the boundary

_CACHE = {}


def build_enc():
    import concourse.bacc as bacc
    import concourse.mybir as mybir
    from concourse import tile

    f32 = mybir.dt.float32
    f8 = mybir.dt.float8e4
    u32 = mybir.dt.uint32
    Act = mybir.ActivationFunctionType
    DR = mybir.MatmulPerfMode.DoubleRow

    nc = bacc.Bacc("TRN2", target_bir_lowering=False, debug=False,
                   num_devices=NCORES)
    xp = nc.dram_tensor("xp", [NBCH, 128, KT2 * 2 * BCH], f8,
                        kind="ExternalInput")
    wp = nc.dram_tensor("wp", [DT, 128, KT2 * 2 * 128], f8,
                        kind="ExternalInput")
    benc = nc.dram_tensor("benc", [DL, 1], f32, kind="ExternalInput")
    cand_v = nc.dram_tensor("cand_v", [128, DT * 8], f32,
                            kind="ExternalOutput")
    cand_i = nc.dram_tensor("cand_i", [128, DT * 8], u32,
                            kind="ExternalOutput")

    benc_r = benc.rearrange("(d p) c -> p (d c)", p=128)

    with tile.TileContext(nc) as tc:
        with (
            tc.tile_pool(name="uni", bufs=1) as unip,
            tc.tile_pool(name="wt", bufs=3) as wtp,
            tc.tile_pool(name="sm", bufs=2) as smp,
            tc.tile_pool(name="ps", bufs=2, space="PSUM") as pse,
        ):
            benc_sb = unip.tile([128, DT], f32, tag="benc", name="benc")
            nc.sync.dma_start(benc_sb[:], benc_r)
            cv = unip.tile([128, DT * 8], f32, tag="cv", name="cv")
            ci = unip.tile([128, DT * 8], u32, tag="ci", name="ci")
            # DMA priority: x chunk 0, weights for d=0/1, then x chunks 1-3
            xts = [unip.tile([128, KT2 * 2 * BCH], f8, tag=f"x{n}",
                             name=f"x{n}") for n in range(NBCH)]
            nc.sync.dma_start(xts[0][:], xp[0, :, :])
            wpre = []
            for d in range(2):
                wth = wtp.tile([128, KT2 * 2 * 128], f8, tag="wt", name="wt")
                nc.sync.dma_start(wth[:], wp[d, :, :])
                wpre.append(wth)
            for n in range(1, NBCH):
                nc.sync.dma_start(xts[n][:], xp[n, :, :])
            xvs = [t[:].rearrange("p (kt ko c) -> p kt ko c", ko=2, c=BCH)
                   for t in xts]
            for d in range(DT):
                if d < 2:
                    wth = wpre[d]
                else:
                    wth = wtp.tile([128, KT2 * 2 * 128], f8, tag="wt",
                                   name="wt")
                    nc.sync.dma_start(wth[:], wp[d, :, :])
                wv = wth[:].rearrange("p (kt ko m) -> p kt ko m",
                                      ko=2, m=128)
                zps = pse.tile([128, B], f32, tag="zps", name="zps")
                if d < 2:
                    # batch-major: start on x chunk 0 before 1-3 arrive
                    order = [(kt, n) for n in range(NBCH)
                             for kt in range(KT2)]
                else:
                    order = [(kt, n) for kt in range(KT2)
                             for n in range(NBCH)]
                for kt, n in order:
                    nc.tensor.matmul(
                        zps[:, n * BCH:(n + 1) * BCH],
                        wv[:, kt], xvs[n][:, kt],
                        start=(kt == 0), stop=(kt == KT2 - 1),
                        perf_mode=DR)
                mv = smp.tile([128, 8], f32, tag="mv", name="mv")
                nc.vector.max(mv[:], zps[:])
                nc.vector.max_index(ci[:, d * 8:(d + 1) * 8], mv[:], zps[:])
                nc.scalar.activation(cv[:, d * 8:(d + 1) * 8], mv[:],
                                     Act.Relu, bias=benc_sb[:, d:d + 1],
                                     scale=1.0 / WSCALE)
                nc.sync.dma_start(cand_v[:, d * 8:(d + 1) * 8],
                                  cv[:, d * 8:(d + 1) * 8])
                nc.sync.dma_start(cand_i[:, d * 8:(d + 1) * 8],
                                  ci[:, d * 8:(d + 1) * 8])
    nc.compile()
    return nc


def build_dec():
    import concourse.bacc as bacc
    import concourse.mybir as mybir
    from concourse import tile

    f32 = mybir.dt.float32
    f16 = mybir.dt.float16

    nc = bacc.Bacc("TRN2", target_bir_lowering=False, debug=False,
                   num_devices=NCORES)
    Gin = nc.dram_tensor("Gin", [NSLOT, AS], f16, kind="ExternalInput")
    Pin = nc.dram_tensor("Pin", [NGRP, 128, DCH], f16, kind="ExternalInput")
    out = nc.dram_tensor("out", [AS, B], f16, kind="ExternalOutput")

    Gin_r = Gin.rearrange("(g p) e -> p g e", p=128)     # [128, NGRP, AS]

    with tile.TileContext(nc) as tc:
        with (
            tc.tile_pool(name="uni", bufs=1) as unip,
            tc.tile_pool(name="sm", bufs=4) as smp,
            tc.tile_pool(name="ps", bufs=2, space="PSUM") as psd,
        ):
            gts, pts = [], []
            for g in range(NGRP):
                gt = unip.tile([128, AS], f16, tag=f"g{g}", name=f"g{g}")
                nc.sync.dma_start(gt[:], Gin_r[:, g, :])
                pt = unip.tile([128, DCH], f16, tag=f"p{g}", name=f"p{g}")
                nc.sync.dma_start(pt[:], Pin[g, :, :])
                gts.append(gt)
                pts.append(pt)
            for n in range(NDCH):
                dpss = [psd.tile([128, DCH], f32, tag=f"dps{at}",
                                 name=f"dps{at}")
                        for at in range(AT)]
                for j in range(NGC):
                    g = n * NGC + j
                    for at in range(AT):
                        nc.tensor.matmul(
                            dpss[at][:],
                            gts[g][:, at * 128:(at + 1) * 128],
                            pts[g][:], start=(j == 0), stop=(j == NGC - 1))
                for at in range(AT):
                    osb = smp.tile([128, DCH], f16, tag="osb", name="osb")
                    nc.vector.tensor_copy(osb[:], dpss[at][:])
                    nc.sync.dma_start(
                        out[at * 128:(at + 1) * 128,
                            n * DCH:(n + 1) * DCH], osb[:])
    nc.compile()
    return nc


def _get_ncs():
    if "enc" not in _CACHE:
        _CACHE["enc"] = build_enc()
        _CACHE["dec"] = build_dec()
    return _CACHE["enc"], _CACHE["dec"]


def _pack_x(xa):
    import ml_dtypes
    x8t = np.ascontiguousarray(xa.T).astype(ml_dtypes.float8_e4m3)  # [A, B]
    arr = x8t.reshape(KT2, 2, 128, B).transpose(2, 0, 1, 3)  # [128,kt,ko,B]
    return np.ascontiguousarray(
        arr.reshape(128, KT2 * 2, B).transpose(0, 2, 1)
        .reshape(128, NBCH, BCH, KT2 * 2).transpose(1, 0, 3, 2))


def _pack_x2(xa):
    import ml_dtypes
    x8t = np.ascontiguousarray(xa.T).astype(ml_dtypes.float8_e4m3)  # [A, B]
    arr = x8t.reshape(KT2, 2, 128, B).transpose(2, 0, 1, 3)  # [128,kt,ko,B]
    return [np.ascontiguousarray(
        arr[:, :, :, n * BCH:(n + 1) * BCH]).reshape(128, KT2 * 2 * BCH)
        for n in range(NBCH)]


def _pack_w(Wc):
    """Wc: [DL, A] fp32 core shard -> [DT, 128, KT2*2*128] fp8 (x64)."""
    import ml_dtypes
    w8t = np.ascontiguousarray(Wc.T * WSCALE).astype(ml_dtypes.float8_e4m3)
    arr = w8t.reshape(KT2, 2, 128, DT, 128).transpose(3, 2, 0, 1, 4)
    return np.ascontiguousarray(arr).reshape(DT, 128, KT2 * 2 * 128)


def kernel(x, W_enc, b_enc, W_dec, b_dec):
    from concourse.bass_utils import run_bass_kernel_spmd

    x = np.asarray(x, np.float32)
    W_enc = np.asarray(W_enc, np.float32)
    b_enc = np.asarray(b_enc, np.float32)
    W_dec = np.asarray(W_dec, np.float32)
    b_dec = np.asarray(b_dec, np.float32)
    nc_enc, nc_dec = _get_ncs()

    xa = x - b_dec[None, :]
    xps = np.stack(_pack_x2(xa))
    in1 = []
    for i in range(NCORES):
        sl = slice(i * DL, (i + 1) * DL)
        in1.append({
            "xp": xps,
            "wp": _pack_w(W_enc[sl]),
            "benc": np.ascontiguousarray(b_enc[sl]).reshape(DL, 1),
        })
    r1 = run_bass_kernel_spmd(nc_enc, in1, core_ids=list(range(NCORES)))

    # ---- host merge: per-dict-row top-8 candidates -> exact global top-K --
    dloc = (np.arange(128)[:, None]
            + 128 * (np.arange(DT * 8)[None, :] // 8))
    cv = np.stack([r1.results[c]["cand_v"] for c in range(NCORES)])
    bi = np.stack([r1.results[c]["cand_i"].astype(np.int64)
                   for c in range(NCORES)])
    dg = (dloc[None, :, :] + (np.arange(NCORES) * DL)[:, None, None])
    cvf, bif, dgf = cv.ravel(), bi.ravel(), dg.ravel()
    kth0 = np.partition(cvf, -K)[-K]
    uni = np.nonzero(cvf >= kth0 - DELTA)[0]
    ub, ud = bif[uni], dgf[uni]
    v32 = (np.einsum("ij,ij->i", W_enc[ud], xa[ub], optimize=True)
           + b_enc[ud])
    kth32 = np.partition(v32, -K)[-K]
    wnd = np.abs(v32 - kth32) <= FP64_WND
    if wnd.any():
        wi = np.nonzero(wnd)[0]
        v32 = v32.astype(np.float64)
        v32[wi] = (np.einsum("ij,ij->i", W_enc[ud[wi]].astype(np.float64),
                             xa[ub[wi]].astype(np.float64))
                   + b_enc[ud[wi]])
    order = np.argsort(-v32)[:K]
    acts = np.maximum(v32[order], 0.0).astype(np.float32)
    rows_b = ub[order]
    cols_d = ud[order]

    # ---- sort by batch row, pack into per-chunk slot groups ----
    srt = np.argsort(rows_b, kind="stable")
    acts, rows_b, cols_d = acts[srt], rows_b[srt], cols_d[srt]
    chunk = rows_b // DCH
    s_acts = np.zeros(NSLOT, np.float32)
    s_rows = np.full(NSLOT, -1.0, np.float32)
    s_cols = np.zeros(NSLOT, np.int64)
    extra = []                       # (row, col, act) applied on host
    for n in range(NDCH):
        idx = np.nonzero(chunk == n)[0]
        if len(idx) > CAPC:
            keep = np.argsort(-acts[idx])[:CAPC]
            drop = np.setdiff1d(np.arange(len(idx)), keep)
            for t in idx[drop]:
                extra.append((rows_b[t], cols_d[t], acts[t]))
            idx = idx[np.sort(keep)]
        base = n * CAPC
        s_acts[base:base + len(idx)] = acts[idx]
        s_rows[base:base + len(idx)] = rows_b[idx].astype(np.float32)
        s_cols[base:base + len(idx)] = cols_d[idx]

    # P[g, p, c] = act of slot g*128+p if its row == chunk(g)*DCH + c
    Wsel = W_dec[s_cols].astype(np.float16)                # [NSLOT, A]
    slot_chunk = (np.arange(NSLOT) // CAPC)
    rel = s_rows - (slot_chunk * DCH).astype(np.float32)   # row within chunk
    Pmat = np.zeros((NSLOT, DCH), np.float16)
    valid = (rel >= 0) & (rel < DCH)
    vi = np.nonzero(valid)[0]
    Pmat[vi, rel[vi].astype(np.int64)] = s_acts[vi].astype(np.float16)
    Pin = np.ascontiguousarray(Pmat.reshape(NGRP, 128, DCH))

    in2 = [{"Gin": np.ascontiguousarray(Wsel[:, c * AS:(c + 1) * AS]),
            "Pin": Pin} for c in range(NCORES)]
    r2 = run_bass_kernel_spmd(nc_dec, in2, core_ids=list(range(NCORES)))

    xhatT = np.empty((A, B), np.float32)
    for c in range(NCORES):
        xhatT[c * AS:(c + 1) * AS, :] = r2.results[c]["out"]
    out = np.ascontiguousarray(xhatT.T) + b_dec[None, :]
    for row, col, act in extra:
        out[row] += act * W_dec[col]
    return out
